# revision 23
# baseline (speedup 1.0000x reference)
"""Fused attention block (QKV conv -> 4-head attention -> proj -> BatchNorm -> LeakyReLU)
distributed over 8 trn2 NeuronCores, data-parallel over the batch dim.

Self-contained: hardcodes shapes B=8, C=64, N=2048, H=4.

Per-core layout tricks (v2 — concurrent PE tiling):
  - scores computed transposed (S^T = K^T Q, keys on partitions); the 4 heads'
    score matmuls go to 4 distinct PE row-groups (tile_position=(32h,0)) and
    are issued back-to-back so they stream CONCURRENTLY through the array;
  - PV matmuls go to 4 distinct col-groups (tile_position=(0,32h)), also
    issued as a concurrent block, accumulating over key tiles in PSUM;
  - exp is split across engines: heads 0,1 take true exp on the scalar
    engine; heads 2,3 take a Schraudolph fast-exp on the vector engine
    (f32->int16 cast of a*s+b, bitcast to fp16), with the affine a*s+b
    folded into the QK matmul via a constants row (k row=1, q row=b;
    a folded into wk host-side);
  - softmax denominators come free from a ones-column in the PV stationary;
    reciprocal via exp(-ln(x)) on the scalar engine (DVE reciprocal is 8x);
  - BatchNorm stats all-reduced across cores ([128,2] f32); a dummy
    AllReduce at prologue warms the CC stream; the channel-half fold +
    replicate is one matmul against a pair-identity matrix;
  - final BN+LeakyReLU is a single Prelu activation with per-partition
    scale/bias.
"""
import numpy as np
import ml_dtypes

import concourse.bass as bass
import concourse.mybir as mybir
from concourse import bacc, tile
from concourse.bass_utils import run_bass_kernel_spmd

B, C, N, H, D = 8, 64, 2048, 4, 16
C2 = 2 * C           # 128 input channels after concat
NC = 512             # query-dim chunk = one fp32 PSUM bank
NCH = N // NC        # 4 chunks
MT = N // 128        # 16 key tiles of 128
F32 = mybir.dt.float32
BF16 = mybir.dt.bfloat16
F16 = mybir.dt.float16
I16 = mybir.dt.int16
SCALE = float(D) ** -0.5
BN_EPS = 1e-5
LEAK = 0.2
N_CORES = 8
CNT = float(B * N)   # batchnorm population count
A_EXP = 1024.0 * 1.4426950408889634   # 2^10 * log2(e)
B_EXP = 15360.0                       # 15 << 10: fp16 exponent bias

Alu = mybir.AluOpType
Act = mybir.ActivationFunctionType


def build():
    nc = bacc.Bacc("TRN2", target_bir_lowering=False, debug=False,
                   num_devices=N_CORES)
    x_p = nc.declare_dram_parameter("x", [C2, N], BF16, isOutput=False)
    wq_p = nc.declare_dram_parameter("wq", [C2, 128], BF16, isOutput=False)
    wk_p = nc.declare_dram_parameter("wk", [C2, 128], BF16, isOutput=False)
    wv_p = nc.declare_dram_parameter("wv", [C2, C], BF16, isOutput=False)
    wp_p = nc.declare_dram_parameter("wp", [C2, C], BF16, isOutput=False)
    g_p = nc.declare_dram_parameter("gamma", [C, 1], F32, isOutput=False)
    b_p = nc.declare_dram_parameter("beta", [C, 1], F32, isOutput=False)
    fold_p = nc.declare_dram_parameter("fold", [C2, 128], F32, isOutput=False)
    out_p = nc.declare_dram_parameter("out", [C, N], F32, isOutput=True)

    with tile.TileContext(nc) as tc:
        with (
            tc.tile_pool(name="sb", bufs=1) as sb,
            tc.tile_pool(name="ps_s", bufs=3, space="PSUM") as ps_s,
            tc.tile_pool(name="ps_pv", bufs=2, space="PSUM") as ps_pv,
            tc.tile_pool(name="pp", bufs=6) as pp,
            tc.tile_pool(name="ep", bufs=2) as ep,
            tc.tile_pool(name="dram", bufs=2, space="DRAM") as dram,
        ):
            # ---- persistent SBUF tiles
            x_sb = sb.tile([C2, N], BF16, tag="x")
            wq_sb = sb.tile([C2, 128], BF16, tag="wq")
            wk_sb = sb.tile([C2, 128], BF16, tag="wk")
            wv_sb = sb.tile([C2, C], BF16, tag="wv")
            wp_sb = sb.tile([C2, C], BF16, tag="wp")
            g_sb = sb.tile([C2, 1], F32, tag="g")     # gamma replicated x2
            b_sb = sb.tile([C2, 1], F32, tag="b")
            fold_sb = sb.tile([C2, 128], F32, tag="fold")
            q_sb = sb.tile([C2, N], BF16, tag="q")    # head h rows 32h..32h+16
            k_sb = sb.tile([C2, N], BF16, tag="k")
            # per key-tile, per head: 32 cols = [16 V^T | 1 ones | 15 zeros]
            vt_sb = sb.tile([C2, MT * 128], F16, tag="vt")
            y_sb = sb.tile([C2, 2 * NC], F32, tag="y")  # proj out, fold layout
            yl_sb = sb.tile([C2, 2 * NC], F32, tag="yl")
            stats = sb.tile([C2, 4], F32, tag="stats")
            dmy_sb = sb.tile([8, 2], F32, tag="dmy")
            eps_t = sb.tile([C2, 1], F32, tag="eps")   # BN_EPS

            # ---- prologue loads, spread across engines' DMA queues; x in
            # 512-col pieces so the first QKV matmul starts after 128KB
            nc.sync.dma_start(x_sb[:, 0:512], x_p[:, 0:512])
            nc.scalar.dma_start(wq_sb[:], wq_p[:])
            nc.scalar.dma_start(wk_sb[:], wk_p[:])
            nc.sync.dma_start(x_sb[:, 512:1024], x_p[:, 512:1024])
            nc.scalar.dma_start(x_sb[:, 1024:1536], x_p[:, 1024:1536])
            nc.sync.dma_start(x_sb[:, 1536:N], x_p[:, 1536:N])
            nc.scalar.dma_start(wv_sb[:], wv_p[:])
            nc.scalar.dma_start(wp_sb[:], wp_p[:])
            nc.gpsimd.dma_start(fold_sb[:], fold_p[:])
            nc.gpsimd.dma_start(g_sb[0:C, :], g_p[:])
            nc.gpsimd.dma_start(g_sb[C:C2, :], g_p[:])
            nc.gpsimd.dma_start(b_sb[0:C, :], b_p[:])
            nc.gpsimd.dma_start(b_sb[C:C2, :], b_p[:])

            # V^T zero fill + ones columns on gpsimd (before it blocks on the
            # warm-up collective)
            nc.gpsimd.memset(eps_t[:], BN_EPS)
            nc.gpsimd.memset(vt_sb[:], 0.0)
            ones_ap = vt_sb[:].rearrange(
                "q (p h e) -> q p h e", p=MT, h=H, e=32)[:, :, :, 16:17]
            nc.gpsimd.memset(ones_ap, 1.0)

            # ---- QKV projections. q/k evacuated with head h at rows
            # 32h..32h+16 (stationary has zeros elsewhere); constant rows
            # 32h+16 are memset afterwards (q row = B_EXP, k row = 1) so the
            # score matmul computes A_EXP*SCALE*(k.q) + B_EXP directly.
            for c4 in range(4):
                cs = slice(512 * c4, 512 * (c4 + 1))
                qp = ps_pv.tile([C2, NC], F32, tag="pv")
                nc.tensor.matmul(qp[:], lhsT=wq_sb[:], rhs=x_sb[:, cs])
                nc.scalar.activation(q_sb[:, cs], qp[:], Act.Copy)
                kp = ps_pv.tile([C2, NC], F32, tag="pv")
                nc.tensor.matmul(kp[:], lhsT=wk_sb[:], rhs=x_sb[:, cs])
                nc.vector.tensor_copy(k_sb[:, cs], kp[:])

            # warm-up AllReduce: wakes the CC stream early so the real one at
            # the tail skips the cold-start latency. gpsimd has no further
            # duties until the tail, so blocking its queue here is free.
            dm_in = dram.tile([8, 2], F32, tag="dm_in")
            dm_out = dram.tile([8, 2], F32, tag="dm_out")
            nc.gpsimd.memset(dmy_sb[:], 1.0)
            nc.gpsimd.dma_start(dm_in[:], dmy_sb[:])
            nc.gpsimd.collective_compute(
                "AllReduce", Alu.add,
                replica_groups=[list(range(N_CORES))],
                ins=[dm_in.opt()], outs=[dm_out.opt()])

            # all 16 V^T key tiles in ONE psum allocation
            vp_all = ps_s.tile([C2, MT * C], F32, tag="s")
            for p in range(MT):
                nc.tensor.matmul(vp_all[:, C * p:C * (p + 1)],
                                 lhsT=x_sb[:, 128 * p:128 * (p + 1)],
                                 rhs=wv_sb[:])
            vt_dst = vt_sb[:].rearrange(
                "q (p h e) -> q p h e", p=MT, h=H, e=32)[:, :, :, 0:16]
            vt_src = vp_all[:].rearrange(
                "q (p h d) -> q p h d", p=MT, h=H, d=D)
            nc.vector.tensor_copy(vt_dst, vt_src)

            def epilogue(c, pv):
                """Normalize chunk-c attention output, project, evac + stats."""
                # softmax denominators: 1/x as exp(-ln(x)) on the scalar
                # engine; only rows 32h+16 (the ones-column sums) are used.
                dln = ep.tile([C2, NC], F32, tag="dln")
                nc.scalar.activation(dln[:], pv[:], Act.Ln)
                drc = ep.tile([C2, NC], F32, tag="drc")
                nc.scalar.activation(drc[:], dln[:], Act.Exp, scale=-1.0)
                rec_d = dram.tile([H, NC], F32, tag="rec_d")
                for h in range(H):
                    eng = nc.sync if h < 2 else nc.scalar
                    eng.dma_start(rec_d[h:h + 1, :],
                                  drc[32 * h + 16:32 * h + 17, :])
                rbc = ep.tile([C2, NC], F32, tag="rbc")
                for h in range(H):
                    eng = nc.sync if h < 2 else nc.scalar
                    eng.dma_start(
                        rbc[32 * h:32 * h + 32, :],
                        rec_d[h:h + 1, :].partition_broadcast(32))
                on = ep.tile([C2, NC], BF16, tag="on")
                nc.vector.tensor_mul(on[:], pv[:], rbc[:])
                yp = ps_s.tile([C2, 2 * NC], F32, tag="s")
                r = slice(64 * (c // 2), 64 * (c // 2) + 64)
                nc.tensor.matmul(yp[r, 0:NC], lhsT=wp_sb[:], rhs=on[:],
                                 tile_position=(0, 64 * (c // 2)))
                ycols = slice(512 * (c % 2), 512 * (c % 2) + 512)
                s0 = 2 * (c % 2)
                nc.vector.tensor_scalar(y_sb[r, ycols], yp[r, 0:NC], 1.0, 0.0,
                                        op0=Alu.mult, op1=Alu.add,
                                        accum_out=stats[r, s0:s0 + 1])
                ysq = ep.tile([C2, NC], F32, tag="dln")
                nc.vector.scalar_tensor_tensor(ysq[r, :], y_sb[r, ycols], 0.0,
                                               y_sb[r, ycols], op0=Alu.add,
                                               op1=Alu.mult,
                                               accum_out=stats[r, s0 + 1:s0 + 2])

            # stats-gather staging (split: top chunk-pair mid-kernel, bottom
            # pair at the tail)
            red_a = sb.tile([C, 2], F32, tag="red_a")
            red_b = sb.tile([C, 2], F32, tag="red_b")
            st_in_a = dram.tile([C, 2], F32, tag="st_in_a")
            st_out_a = dram.tile([N_CORES * C, 2], F32, tag="st_out_a")
            st_in_b = dram.tile([C, 2], F32, tag="st_in_b")
            st_out_b = dram.tile([N_CORES * C, 2], F32, tag="st_out_b")

            # ---- attention: per (chunk, key-tile): 4 concurrent row-tiled
            # score MMs; exp split scalar/vector; 4 concurrent col-tiled PV
            # MMs lagged one tile so the PE never waits on exp.
            prev = None
            for c in range(NCH):
                pv = ps_pv.tile([C2, NC], F32, tag="pv")
                qs = slice(NC * c, NC * (c + 1))
                pend = None

                def flush(pv=pv):
                    nonlocal pend
                    if pend is None:
                        return
                    p01, p23, t = pend
                    pend = None
                    for h in range(H):
                        src = p01 if h < 2 else p23
                        nc.tensor.matmul(
                            pv[32 * h:32 * h + 32, :],
                            lhsT=vt_sb[:, 128 * t + 32 * h:128 * t + 32 * h + 32],
                            rhs=src[:, 512 * (h % 2):512 * (h % 2) + 512],
                            start=(t == 0), stop=(t == MT - 1),
                            skip_group_check=True,
                            tile_position=(0, 32 * h))

                for t in range(MT):
                    if c > 0 and t == 2 and prev is not None:
                        epilogue(*prev)
                        prev = None
                        if c == 2:
                            # chunks 0,1 stats (rows 0:64) are final: gather
                            # them now, hidden under chunks 2,3 compute
                            nc.vector.tensor_add(red_a[:], stats[0:C, 0:2],
                                                 stats[0:C, 2:4])
                            nc.gpsimd.dma_start(st_in_a[:], red_a[:])
                            nc.gpsimd.collective_compute(
                                "AllGather", Alu.bypass,
                                replica_groups=[list(range(N_CORES))],
                                ins=[st_in_a.opt()], outs=[st_out_a.opt()])
                    # pool rotation note: with 3 slots and 2 tiles/t, the
                    # sp01-first order gives DVE-freed slots two tiles of
                    # slack and scalar-freed slots one — measured fastest.
                    sp01 = ps_s.tile([C2, 1024], F32, tag="s")
                    sp23 = ps_s.tile([C2, 1024], F32, tag="s")
                    for h in range(H):
                        sp = sp01 if h < 2 else sp23
                        nc.tensor.matmul(
                            sp[:, 512 * (h % 2):512 * (h % 2) + 512],
                            lhsT=k_sb[32 * h:32 * h + 16, 128 * t:128 * (t + 1)],
                            rhs=q_sb[32 * h:32 * h + 16, qs],
                            tile_position=(32 * h, 0))
                    # exp now (other engines), PV one tile behind
                    p01 = pp.tile([C2, 1024], F16, tag="p")
                    nc.scalar.activation(p01[:], sp01[:], Act.Exp,
                                         scale=1.0 / A_EXP)
                    p23 = pp.tile([C2, 1024], F16, tag="p")
                    nc.vector.tensor_scalar(p23[:].bitcast(I16), sp23[:],
                                            1.0, B_EXP,
                                            op0=Alu.mult, op1=Alu.add)
                    flush()
                    pend = (p01, p23, t)
                flush()
                prev = (c, pv)
            epilogue(*prev)

            # ---- gather the bottom chunk-pair stats (rows 64:128); scalar
            # engine triggers (gpsimd's semaphore wake costs ~3us)
            nc.vector.tensor_add(red_b[:], stats[C:C2, 0:2], stats[C:C2, 2:4])
            nc.scalar.dma_start(st_in_b[:], red_b[:])
            nc.gpsimd.collective_compute(
                "AllGather", Alu.bypass,
                replica_groups=[list(range(N_CORES))],
                ins=[st_in_b.opt()], outs=[st_out_b.opt()])
            # pull the 8 cores' [64,2] blocks side by side: [128, 8*2]
            ag = sb.tile([C2, 2 * N_CORES], F32, tag="ag")
            ag_src_a = st_out_a[:].rearrange("(i p) s -> p i s", i=N_CORES)
            nc.sync.dma_start(
                ag[0:C, :].rearrange("p (i s) -> p i s", i=N_CORES), ag_src_a)
            ag_src_b = st_out_b[:].rearrange("(i p) s -> p i s", i=N_CORES)
            nc.sync.dma_start(
                ag[C:C2, :].rearrange("p (i s) -> p i s", i=N_CORES), ag_src_b)
            ag8 = sb.tile([C2, 8], F32, tag="ag8")
            nc.vector.tensor_add(ag8[:], ag[:, 0:8], ag[:, 8:16])
            ag4 = sb.tile([C2, 4], F32, tag="ag4")
            nc.vector.tensor_add(ag4[:], ag8[:, 0:4], ag8[:, 4:8])
            fa = sb.tile([C2, 2], F32, tag="fa")
            nc.vector.tensor_add(fa[:], ag4[:, 0:2], ag4[:, 2:4])
            # fold the channel halves + replicate to 128 partitions in one
            # matmul against the pair-identity matrix
            fps = ps_pv.tile([C2, NC], F32, tag="pv")
            nc.tensor.matmul(fps[:, 0:2], lhsT=fold_sb[:], rhs=fa[:])

            # ---- finalize: mean/var -> scale/shift, all [128, *] replicated
            ms = sb.tile([C2, 2], F32, tag="ms")
            nc.vector.tensor_scalar_mul(ms[:], fps[:, 0:2], 1.0 / CNT)
            msq = sb.tile([C2, 1], F32, tag="msq")
            nc.vector.tensor_mul(msq[:], ms[:, 0:1], ms[:, 0:1])
            var = sb.tile([C2, 1], F32, tag="var")
            nc.vector.tensor_sub(var[:], ms[:, 1:2], msq[:])
            lnv = sb.tile([C2, 1], F32, tag="lnv")
            nc.scalar.activation(lnv[:], var[:], Act.Ln, bias=eps_t[:, 0:1])
            istd = sb.tile([C2, 1], F32, tag="istd")
            nc.scalar.activation(istd[:], lnv[:], Act.Exp, scale=-0.5)
            sc = sb.tile([C2, 1], F32, tag="sc")
            nc.vector.tensor_mul(sc[:], g_sb[:], istd[:])
            msc = sb.tile([C2, 1], F32, tag="msc")
            nc.vector.tensor_scalar(msc[:], ms[:, 0:1], sc[:, 0:1], None,
                                    op0=Alu.mult)
            sh = sb.tile([C2, 1], F32, tag="sh")
            nc.vector.tensor_sub(sh[:], b_sb[:], msc[:])

            # ---- BN scale/shift + LeakyReLU in one Prelu pass + store
            nc.scalar.activation(yl_sb[:], y_sb[:], Act.Prelu,
                                 scale=sc[:, 0:1], bias=sh[:, 0:1], alpha=LEAK)
            nc.sync.dma_start(out_p[:, 0:1024], yl_sb[0:C, :])
            nc.sync.dma_start(out_p[:, 1024:N], yl_sb[C:C2, :])

    nc.compile()

    # Post-compile surgery: one activation table set covers every function
    # used here (Exp, Ln, Copy, Prelu); point the first load at it and drop
    # the rest so the table-load inserter doesn't ping-pong.
    from concourse.hw_specs import get_activation_tables
    tabs = list(get_activation_tables(nc.m.arch).keys())
    nle = tabs.index("natural_log_exp_and_others")
    loads = [(b, i) for b in nc.main_func.blocks for i in b.instructions
             if isinstance(i, mybir.InstLoadActFuncSet)]
    if loads:
        loads[0][1].act_func_set_id = nle
        for b, i in loads[1:]:
            b.instructions.remove(i)
    return nc


_NC_CACHE = None


def _get_nc():
    global _NC_CACHE
    if _NC_CACHE is None:
        _NC_CACHE = build()
    return _NC_CACHE


def _prep_inputs(x_local, x_branch, w_qkv, w_proj, gamma, beta):
    bf16 = ml_dtypes.bfloat16
    x_local = np.asarray(x_local, np.float32)
    x_branch = np.asarray(x_branch, np.float32)
    w_qkv = np.asarray(w_qkv, np.float32)
    w_proj = np.asarray(w_proj, np.float32)
    gamma = np.asarray(gamma, np.float32)
    beta = np.asarray(beta, np.float32)

    X = np.concatenate([x_local, x_branch], axis=1).astype(bf16)  # [B, 128, N]
    WT = w_qkv.T.copy()  # [128, 192]
    wq = np.zeros((C2, 128), np.float32)
    wk = np.zeros((C2, 128), np.float32)
    for h in range(H):
        wq[:, 32 * h:32 * h + D] = WT[:, D * h:D * (h + 1)]
        # fold the softmax scale and the Schraudolph slope into wk
        wk[:, 32 * h:32 * h + D] = WT[:, C + D * h:C + D * (h + 1)] * (A_EXP * SCALE)
    wv = WT[:, 2 * C:3 * C]
    wp = np.zeros((C2, C), np.float32)
    for h in range(H):
        wp[32 * h:32 * h + D, :] = w_proj[:, D * h:D * (h + 1)].T
    fold = np.zeros((C2, 128), np.float32)
    for r in range(C2):
        fold[r, r % 64] = 1.0
        fold[r, r % 64 + 64] = 1.0
    common = dict(
        wq=wq.astype(bf16), wk=wk.astype(bf16),
        wv=np.ascontiguousarray(wv).astype(bf16),
        wp=wp.astype(bf16),
        fold=fold,
        gamma=np.ascontiguousarray(gamma.reshape(C, 1)),
        beta=np.ascontiguousarray(beta.reshape(C, 1)),
    )
    return [dict(x=np.ascontiguousarray(X[b]), **common) for b in range(B)]


def kernel(x_local, x_branch, w_qkv, w_proj, gamma, beta, _trace=False, _tmpdir=None):
    nc = _get_nc()
    in_maps = _prep_inputs(x_local, x_branch, w_qkv, w_proj, gamma, beta)
    res = run_bass_kernel_spmd(nc, in_maps, core_ids=list(range(N_CORES)),
                               trace=_trace, tmpdir=_tmpdir)
    out = np.stack([np.asarray(res.results[i]["out"]) for i in range(N_CORES)])
    if _trace:
        kernel._last_results = res
    return out.astype(np.float32)


# revision 27
# speedup vs baseline: 1.2615x; 1.2615x over previous
"""Fused attention block (QKV conv -> 4-head attention -> proj -> BatchNorm -> LeakyReLU)
distributed over 8 trn2 NeuronCores, data-parallel over the batch dim.

Self-contained: hardcodes shapes B=8, C=64, N=2048, H=4.

Per-core layout tricks (v2 — concurrent PE tiling):
  - scores computed transposed (S^T = K^T Q, keys on partitions); the 4 heads'
    score matmuls go to 4 distinct PE row-groups (tile_position=(32h,0)) and
    are issued back-to-back so they stream CONCURRENTLY through the array;
  - PV matmuls go to 4 distinct col-groups (tile_position=(0,32h)), also
    issued as a concurrent block, accumulating over key tiles in PSUM;
  - exp is split across engines: heads 0,1 take true exp on the scalar
    engine; heads 2,3 take a Schraudolph fast-exp on the vector engine
    (f32->int16 cast of a*s+b, bitcast to fp16), with the affine a*s+b
    folded into the QK matmul via a constants row (k row=1, q row=b;
    a folded into wk host-side);
  - softmax denominators come free from a ones-column in the PV stationary;
    reciprocal via exp(-ln(x)) on the scalar engine (DVE reciprocal is 8x);
  - BatchNorm stats all-reduced across cores ([128,2] f32); a dummy
    AllReduce at prologue warms the CC stream; the channel-half fold +
    replicate is one matmul against a pair-identity matrix;
  - final BN+LeakyReLU is a single Prelu activation with per-partition
    scale/bias.
"""
import numpy as np
import ml_dtypes

import concourse.bass as bass
import concourse.mybir as mybir
from concourse import bacc, tile
from concourse.bass_utils import run_bass_kernel_spmd

B, C, N, H, D = 8, 64, 2048, 4, 16
C2 = 2 * C           # 128 input channels after concat
NC = 512             # query-dim chunk = one fp32 PSUM bank
NCH = N // NC        # 4 chunks
MT = N // 128        # 16 key tiles of 128
F32 = mybir.dt.float32
BF16 = mybir.dt.bfloat16
F16 = mybir.dt.float16
I16 = mybir.dt.int16
SCALE = float(D) ** -0.5
BN_EPS = 1e-5
LEAK = 0.2
N_CORES = 8
CNT = float(B * N)   # batchnorm population count
A_EXP = 1024.0 * 1.4426950408889634   # 2^10 * log2(e)
B_EXP = 15360.0                       # 15 << 10: fp16 exponent bias

Alu = mybir.AluOpType
Act = mybir.ActivationFunctionType


def build():
    nc = bacc.Bacc("TRN2", target_bir_lowering=False, debug=False,
                   num_devices=N_CORES)
    x_p = nc.declare_dram_parameter("x", [C2, N], BF16, isOutput=False)
    wq_p = nc.declare_dram_parameter("wq", [C2, 128], BF16, isOutput=False)
    wk_p = nc.declare_dram_parameter("wk", [C2, 128], BF16, isOutput=False)
    wv_p = nc.declare_dram_parameter("wv", [C2, C], BF16, isOutput=False)
    wp_p = nc.declare_dram_parameter("wp", [C2, C], BF16, isOutput=False)
    g_p = nc.declare_dram_parameter("gamma", [C, 1], F32, isOutput=False)
    b_p = nc.declare_dram_parameter("beta", [C, 1], F32, isOutput=False)
    fold_p = nc.declare_dram_parameter("fold", [C2, 128], F32, isOutput=False)
    out_p = nc.declare_dram_parameter("out", [C, N], F32, isOutput=True)

    with tile.TileContext(nc) as tc:
        with (
            tc.tile_pool(name="sb", bufs=1) as sb,
            tc.tile_pool(name="ps_s", bufs=3, space="PSUM") as ps_s,
            tc.tile_pool(name="ps_pv", bufs=2, space="PSUM") as ps_pv,
            tc.tile_pool(name="pp", bufs=6) as pp,
            tc.tile_pool(name="ep", bufs=2) as ep,
            tc.tile_pool(name="dram", bufs=2, space="DRAM") as dram,
        ):
            # ---- persistent SBUF tiles
            x_sb = sb.tile([C2, N], BF16, tag="x")
            wq_sb = sb.tile([C2, 128], BF16, tag="wq")
            wk_sb = sb.tile([C2, 128], BF16, tag="wk")
            wv_sb = sb.tile([C2, C], BF16, tag="wv")
            wp_sb = sb.tile([C2, C], BF16, tag="wp")
            g_sb = sb.tile([C2, 1], F32, tag="g")     # gamma replicated x2
            b_sb = sb.tile([C2, 1], F32, tag="b")
            fold_sb = sb.tile([C2, 128], F32, tag="fold")
            q_sb = sb.tile([C2, N], BF16, tag="q")    # head h rows 32h..32h+16
            k_sb = sb.tile([C2, N], BF16, tag="k")
            # per key-tile, per head: 32 cols = [16 V^T | 1 ones | 15 zeros]
            vt_sb = sb.tile([C2, MT * 128], F16, tag="vt")
            y_sb = sb.tile([C2, 2 * NC], F32, tag="y")  # proj out, fold layout
            yl_sb = sb.tile([C2, 2 * NC], F32, tag="yl")
            stats = sb.tile([C2, 4], F32, tag="stats")
            dmy_sb = sb.tile([8, 2], F32, tag="dmy")
            eps_t = sb.tile([C2, 1], F32, tag="eps")   # BN_EPS

            # ---- prologue loads, spread across engines' DMA queues; x in
            # 512-col pieces so the first QKV matmul starts after 128KB
            nc.sync.dma_start(x_sb[:, 0:512], x_p[:, 0:512])
            nc.scalar.dma_start(wq_sb[:], wq_p[:])
            nc.scalar.dma_start(wk_sb[:], wk_p[:])
            nc.sync.dma_start(x_sb[:, 512:1024], x_p[:, 512:1024])
            nc.scalar.dma_start(x_sb[:, 1024:1536], x_p[:, 1024:1536])
            nc.sync.dma_start(x_sb[:, 1536:N], x_p[:, 1536:N])
            nc.scalar.dma_start(wv_sb[:], wv_p[:])
            nc.scalar.dma_start(wp_sb[:], wp_p[:])
            nc.gpsimd.dma_start(fold_sb[:], fold_p[:])
            nc.gpsimd.dma_start(g_sb[0:C, :], g_p[:])
            nc.gpsimd.dma_start(g_sb[C:C2, :], g_p[:])
            nc.gpsimd.dma_start(b_sb[0:C, :], b_p[:])
            nc.gpsimd.dma_start(b_sb[C:C2, :], b_p[:])

            # V^T zero fill + ones columns on gpsimd (before it blocks on the
            # warm-up collective)
            nc.gpsimd.memset(eps_t[:], BN_EPS)
            nc.gpsimd.memset(vt_sb[:], 0.0)
            ones_ap = vt_sb[:].rearrange(
                "q (p h e) -> q p h e", p=MT, h=H, e=32)[:, :, :, 16:17]
            nc.gpsimd.memset(ones_ap, 1.0)

            # ---- QKV projections. q/k evacuated with head h at rows
            # 32h..32h+16 (stationary has zeros elsewhere); constant rows
            # 32h+16 are memset afterwards (q row = B_EXP, k row = 1) so the
            # score matmul computes A_EXP*SCALE*(k.q) + B_EXP directly.
            for c4 in range(4):
                cs = slice(512 * c4, 512 * (c4 + 1))
                qp = ps_pv.tile([C2, NC], F32, tag="pv")
                nc.tensor.matmul(qp[:], lhsT=wq_sb[:], rhs=x_sb[:, cs])
                nc.scalar.activation(q_sb[:, cs], qp[:], Act.Copy)
                kp = ps_pv.tile([C2, NC], F32, tag="pv")
                nc.tensor.matmul(kp[:], lhsT=wk_sb[:], rhs=x_sb[:, cs])
                nc.vector.tensor_copy(k_sb[:, cs], kp[:])

            # warm-up AllReduce: wakes the CC stream early so the real one at
            # the tail skips the cold-start latency. gpsimd has no further
            # duties until the tail, so blocking its queue here is free.
            dm_in = dram.tile([8, 2], F32, tag="dm_in")
            dm_out = dram.tile([8, 2], F32, tag="dm_out")
            nc.gpsimd.memset(dmy_sb[:], 1.0)
            nc.gpsimd.dma_start(dm_in[:], dmy_sb[:])
            nc.gpsimd.collective_compute(
                "AllReduce", Alu.add,
                replica_groups=[list(range(N_CORES))],
                ins=[dm_in.opt()], outs=[dm_out.opt()])

            # all 16 V^T key tiles in ONE psum allocation
            vp_all = ps_s.tile([C2, MT * C], F32, tag="s")
            for p in range(MT):
                nc.tensor.matmul(vp_all[:, C * p:C * (p + 1)],
                                 lhsT=x_sb[:, 128 * p:128 * (p + 1)],
                                 rhs=wv_sb[:])
            vt_dst = vt_sb[:].rearrange(
                "q (p h e) -> q p h e", p=MT, h=H, e=32)[:, :, :, 0:16]
            vt_src = vp_all[:].rearrange(
                "q (p h d) -> q p h d", p=MT, h=H, d=D)
            nc.vector.tensor_copy(vt_dst, vt_src)

            def epilogue_pre(c, pv):
                """Denominator chain for chunk c: ln/exp + DRAM bounce. Emitted
                at the start of the next chunk so the round-trip latency hides
                behind compute before epilogue_post consumes rbc."""
                # softmax denominators: 1/x as exp(-ln(x)) on the scalar
                # engine; only rows 32h+16 (the ones-column sums) are used.
                dln = ep.tile([C2, NC], F32, tag="dln")
                nc.scalar.activation(dln[:], pv[:], Act.Ln)
                drc = ep.tile([C2, NC], F32, tag="drc")
                nc.scalar.activation(drc[:], dln[:], Act.Exp, scale=-1.0)
                rec_d = dram.tile([H, NC], F32, tag="rec_d")
                for h in range(H):
                    nc.sync.dma_start(rec_d[h:h + 1, :],
                                      drc[32 * h + 16:32 * h + 17, :])
                rbc = ep.tile([C2, NC], F32, tag="rbc")
                for h in range(H):
                    nc.sync.dma_start(
                        rbc[32 * h:32 * h + 32, :],
                        rec_d[h:h + 1, :].partition_broadcast(32))
                return rbc

            def epilogue_post(c, pv, rbc):
                """Normalize chunk-c attention output, project, evac + stats."""
                on = ep.tile([C2, NC], BF16, tag="on")
                nc.vector.tensor_mul(on[:], pv[:], rbc[:])
                yp = ps_s.tile([C2, 2 * NC], F32, tag="s")
                r = slice(64 * (c // 2), 64 * (c // 2) + 64)
                nc.tensor.matmul(yp[r, 0:NC], lhsT=wp_sb[:], rhs=on[:],
                                 tile_position=(0, 64 * (c // 2)))
                ycols = slice(512 * (c % 2), 512 * (c % 2) + 512)
                s0 = 2 * (c % 2)
                nc.vector.tensor_scalar(y_sb[r, ycols], yp[r, 0:NC], 1.0, 0.0,
                                        op0=Alu.mult, op1=Alu.add,
                                        accum_out=stats[r, s0:s0 + 1])
                ysq = ep.tile([C2, NC], F32, tag="dln")
                nc.vector.scalar_tensor_tensor(ysq[r, :], y_sb[r, ycols], 0.0,
                                               y_sb[r, ycols], op0=Alu.add,
                                               op1=Alu.mult,
                                               accum_out=stats[r, s0 + 1:s0 + 2])

            # stats-gather staging (split: top chunk-pair mid-kernel, bottom
            # pair at the tail)
            red_a = sb.tile([C, 2], F32, tag="red_a")
            red_b = sb.tile([C, 2], F32, tag="red_b")
            st_in_a = dram.tile([C, 2], F32, tag="st_in_a")
            st_out_a = dram.tile([N_CORES * C, 2], F32, tag="st_out_a")
            st_in_b = dram.tile([C, 2], F32, tag="st_in_b")
            st_out_b = dram.tile([N_CORES * C, 2], F32, tag="st_out_b")

            # ---- attention: per (chunk, key-tile): 4 concurrent row-tiled
            # score MMs; exp split scalar/vector; 4 concurrent col-tiled PV
            # MMs lagged one tile so the PE never waits on exp.
            prev = None
            prev_rbc = None
            for c in range(NCH):
                pv = ps_pv.tile([C2, NC], F32, tag="pv")
                qs = slice(NC * c, NC * (c + 1))
                pend = None
                if prev is not None:
                    prev_rbc = epilogue_pre(*prev)

                def flush(pv=pv):
                    nonlocal pend
                    if pend is None:
                        return
                    p01, p23, t = pend
                    pend = None
                    for h in range(H):
                        src = p01 if h < 2 else p23
                        nc.tensor.matmul(
                            pv[32 * h:32 * h + 32, :],
                            lhsT=vt_sb[:, 128 * t + 32 * h:128 * t + 32 * h + 32],
                            rhs=src[:, 512 * (h % 2):512 * (h % 2) + 512],
                            start=(t == 0), stop=(t == MT - 1),
                            skip_group_check=True,
                            tile_position=(0, 32 * h))

                for t in range(MT):
                    if c > 0 and t == 8 and prev is not None:
                        epilogue_post(*prev, prev_rbc)
                        prev = None
                        if c == 2:
                            # chunks 0,1 stats (rows 0:64) are final: gather
                            # them now, hidden under chunks 2,3 compute
                            nc.vector.tensor_add(red_a[:], stats[0:C, 0:2],
                                                 stats[0:C, 2:4])
                            nc.gpsimd.dma_start(st_in_a[:], red_a[:])
                            nc.gpsimd.collective_compute(
                                "AllGather", Alu.bypass,
                                replica_groups=[list(range(N_CORES))],
                                ins=[st_in_a.opt()], outs=[st_out_a.opt()])
                    # pool rotation note: with 3 slots and 2 tiles/t, the
                    # sp01-first order gives DVE-freed slots two tiles of
                    # slack and scalar-freed slots one — measured fastest.
                    sp01 = ps_s.tile([C2, 1024], F32, tag="s")
                    sp23 = ps_s.tile([C2, 1024], F32, tag="s")
                    for h in range(H):
                        sp = sp01 if h < 2 else sp23
                        nc.tensor.matmul(
                            sp[:, 512 * (h % 2):512 * (h % 2) + 512],
                            lhsT=k_sb[32 * h:32 * h + 16, 128 * t:128 * (t + 1)],
                            rhs=q_sb[32 * h:32 * h + 16, qs],
                            tile_position=(32 * h, 0))
                    # exp now (other engines), PV one tile behind
                    p01 = pp.tile([C2, 1024], F16, tag="p")
                    nc.scalar.activation(p01[:], sp01[:], Act.Exp,
                                         scale=1.0 / A_EXP)
                    p23 = pp.tile([C2, 1024], F16, tag="p")
                    nc.vector.tensor_scalar(p23[:].bitcast(I16), sp23[:],
                                            1.0, B_EXP,
                                            op0=Alu.mult, op1=Alu.add)
                    flush()
                    pend = (p01, p23, t)
                flush()
                prev = (c, pv)
            prev_rbc = epilogue_pre(*prev)
            epilogue_post(*prev, prev_rbc)

            # ---- gather the bottom chunk-pair stats (rows 64:128); scalar
            # engine triggers (gpsimd's semaphore wake costs ~3us)
            nc.vector.tensor_add(red_b[:], stats[C:C2, 0:2], stats[C:C2, 2:4])
            nc.scalar.dma_start(st_in_b[:], red_b[:])
            nc.gpsimd.collective_compute(
                "AllGather", Alu.bypass,
                replica_groups=[list(range(N_CORES))],
                ins=[st_in_b.opt()], outs=[st_out_b.opt()])
            # pull the 8 cores' [64,2] blocks side by side: [128, 8*2]
            ag = sb.tile([C2, 2 * N_CORES], F32, tag="ag")
            ag_src_a = st_out_a[:].rearrange("(i p) s -> p i s", i=N_CORES)
            nc.sync.dma_start(
                ag[0:C, :].rearrange("p (i s) -> p i s", i=N_CORES), ag_src_a)
            ag_src_b = st_out_b[:].rearrange("(i p) s -> p i s", i=N_CORES)
            nc.sync.dma_start(
                ag[C:C2, :].rearrange("p (i s) -> p i s", i=N_CORES), ag_src_b)
            ag8 = sb.tile([C2, 8], F32, tag="ag8")
            nc.vector.tensor_add(ag8[:], ag[:, 0:8], ag[:, 8:16])
            ag4 = sb.tile([C2, 4], F32, tag="ag4")
            nc.vector.tensor_add(ag4[:], ag8[:, 0:4], ag8[:, 4:8])
            fa = sb.tile([C2, 2], F32, tag="fa")
            nc.vector.tensor_add(fa[:], ag4[:, 0:2], ag4[:, 2:4])
            # fold the channel halves + replicate to 128 partitions in one
            # matmul against the pair-identity matrix
            fps = ps_pv.tile([C2, NC], F32, tag="pv")
            nc.tensor.matmul(fps[:, 0:2], lhsT=fold_sb[:], rhs=fa[:])

            # ---- finalize: mean/var -> scale/shift, all [128, *] replicated
            ms = sb.tile([C2, 2], F32, tag="ms")
            nc.vector.tensor_scalar_mul(ms[:], fps[:, 0:2], 1.0 / CNT)
            msq = sb.tile([C2, 1], F32, tag="msq")
            nc.vector.tensor_mul(msq[:], ms[:, 0:1], ms[:, 0:1])
            var = sb.tile([C2, 1], F32, tag="var")
            nc.vector.tensor_sub(var[:], ms[:, 1:2], msq[:])
            lnv = sb.tile([C2, 1], F32, tag="lnv")
            nc.scalar.activation(lnv[:], var[:], Act.Ln, bias=eps_t[:, 0:1])
            istd = sb.tile([C2, 1], F32, tag="istd")
            nc.scalar.activation(istd[:], lnv[:], Act.Exp, scale=-0.5)
            sc = sb.tile([C2, 1], F32, tag="sc")
            nc.vector.tensor_mul(sc[:], g_sb[:], istd[:])
            msc = sb.tile([C2, 1], F32, tag="msc")
            nc.vector.tensor_scalar(msc[:], ms[:, 0:1], sc[:, 0:1], None,
                                    op0=Alu.mult)
            sh = sb.tile([C2, 1], F32, tag="sh")
            nc.vector.tensor_sub(sh[:], b_sb[:], msc[:])

            # ---- BN scale/shift + LeakyReLU in one Prelu pass + store
            nc.scalar.activation(yl_sb[:], y_sb[:], Act.Prelu,
                                 scale=sc[:, 0:1], bias=sh[:, 0:1], alpha=LEAK)
            nc.sync.dma_start(out_p[:, 0:1024], yl_sb[0:C, :])
            nc.sync.dma_start(out_p[:, 1024:N], yl_sb[C:C2, :])

    nc.compile()

    # Post-compile surgery: one activation table set covers every function
    # used here (Exp, Ln, Copy, Prelu); point the first load at it and drop
    # the rest so the table-load inserter doesn't ping-pong.
    from concourse.hw_specs import get_activation_tables
    tabs = list(get_activation_tables(nc.m.arch).keys())
    nle = tabs.index("natural_log_exp_and_others")
    loads = [(b, i) for b in nc.main_func.blocks for i in b.instructions
             if isinstance(i, mybir.InstLoadActFuncSet)]
    if loads:
        loads[0][1].act_func_set_id = nle
        for b, i in loads[1:]:
            b.instructions.remove(i)
    return nc


_NC_CACHE = None


def _get_nc():
    global _NC_CACHE
    if _NC_CACHE is None:
        _NC_CACHE = build()
    return _NC_CACHE


def _prep_inputs(x_local, x_branch, w_qkv, w_proj, gamma, beta):
    bf16 = ml_dtypes.bfloat16
    x_local = np.asarray(x_local, np.float32)
    x_branch = np.asarray(x_branch, np.float32)
    w_qkv = np.asarray(w_qkv, np.float32)
    w_proj = np.asarray(w_proj, np.float32)
    gamma = np.asarray(gamma, np.float32)
    beta = np.asarray(beta, np.float32)

    X = np.concatenate([x_local, x_branch], axis=1).astype(bf16)  # [B, 128, N]
    WT = w_qkv.T.copy()  # [128, 192]
    wq = np.zeros((C2, 128), np.float32)
    wk = np.zeros((C2, 128), np.float32)
    for h in range(H):
        wq[:, 32 * h:32 * h + D] = WT[:, D * h:D * (h + 1)]
        # fold the softmax scale and the Schraudolph slope into wk
        wk[:, 32 * h:32 * h + D] = WT[:, C + D * h:C + D * (h + 1)] * (A_EXP * SCALE)
    wv = WT[:, 2 * C:3 * C]
    wp = np.zeros((C2, C), np.float32)
    for h in range(H):
        wp[32 * h:32 * h + D, :] = w_proj[:, D * h:D * (h + 1)].T
    fold = np.zeros((C2, 128), np.float32)
    for r in range(C2):
        fold[r, r % 64] = 1.0
        fold[r, r % 64 + 64] = 1.0
    common = dict(
        wq=wq.astype(bf16), wk=wk.astype(bf16),
        wv=np.ascontiguousarray(wv).astype(bf16),
        wp=wp.astype(bf16),
        fold=fold,
        gamma=np.ascontiguousarray(gamma.reshape(C, 1)),
        beta=np.ascontiguousarray(beta.reshape(C, 1)),
    )
    return [dict(x=np.ascontiguousarray(X[b]), **common) for b in range(B)]


def kernel(x_local, x_branch, w_qkv, w_proj, gamma, beta, _trace=False, _tmpdir=None):
    nc = _get_nc()
    in_maps = _prep_inputs(x_local, x_branch, w_qkv, w_proj, gamma, beta)
    res = run_bass_kernel_spmd(nc, in_maps, core_ids=list(range(N_CORES)),
                               trace=_trace, tmpdir=_tmpdir)
    out = np.stack([np.asarray(res.results[i]["out"]) for i in range(N_CORES)])
    if _trace:
        kernel._last_results = res
    return out.astype(np.float32)
